# revision 5
# baseline (speedup 1.0000x reference)
"""LSTM encoder with EOS-freeze for Trainium2, data-parallel over batch on 8 cores.

Strategy (v2)
-------------
Inputs are one-hot, so x @ Wi is a row-gather of Wi done with indirect DMA on
device; rows for two consecutive steps are gathered into one [32, 2048] tile so
one LDWEIGHTS transposes both steps (two matmuls against stacked identities).

The recurrent h @ Wh runs with Wh as 64 fp8-e4m3 [128,128] stationary tiles
(FWL loads fp8 2x faster than fp16) and h chunks as the [128,16] moving
operand. Weights are pre-scaled by 512 on host; the activation instructions
fold 1/512 back in via their scale port.

Gates are split into FOUR separate PSUM tiles (g, i, f, o) in separate banks,
each closed by its own `stop` matmul, so every sigmoid/tanh fires as soon as
its own region finishes instead of waiting for the whole z. Matmul region
order g, i, f, o leaves only sigmoid(o) -> h on the post-block critical path.

The EOS freeze is handled without any per-step masking: sequences are
independent, so the kernel runs the unmasked recurrence and streams per-step
(c, h) snapshots to DRAM; the frozen value for sequence b is the snapshot at
its first-EOS step, selected during unshard.
"""

import numpy as np

try:
    import concourse  # noqa: F401
except ImportError:
    import sys

    sys.path.insert(0, "/opt/trn_rl_repo")

from contextlib import ExitStack

import ml_dtypes

import concourse.bass as bass
import concourse.tile as tile
from concourse import bacc
from concourse import mybir
from concourse.bass import ds
from concourse.bass_utils import run_bass_kernel_spmd

dt = mybir.dt
Alu = mybir.AluOpType
Act = mybir.ActivationFunctionType

EOS_ID = 1
HID = 512
BATCH, SEQ, VOCAB = 128, 256, 1024
GATES = 4 * HID  # 2048
NCORES = 8
BLOC = BATCH // NCORES  # 16 sequences per core
BODY = 16  # steps per For_i iteration
PAIRS = BODY // 2  # x-gather/transpose granularity: 2 steps share one LDW
FP8 = True
WSCALE = 512.0  # host pre-scale on Wi/Wh; activations apply 1/WSCALE

# Collect profiling info when True (set by test.py; adds trace overhead).
TRACE = False
LAST_RESULTS = None  # BassKernelResults of the last run, for test.py

_PROGRAM = None

WH_DT = dt.float8e4 if FP8 else dt.float16
WH_NP = ml_dtypes.float8_e4m3fn if FP8 else np.float16


def _build_program(seq=SEQ, body=BODY):
    pairs = body // 2
    nc = bacc.Bacc("TRN2", debug=False, detect_race_conditions=False)

    wi = nc.declare_dram_parameter("wi", [VOCAB, GATES], dt.float16, isOutput=False)
    ident = nc.declare_dram_parameter("ident", [32, 32], dt.float16, isOutput=False)
    wh = nc.declare_dram_parameter("wh", [128, 64 * 128], WH_DT, isOutput=False)
    tok2 = nc.declare_dram_parameter(
        "tok2", [32, seq // 2 + pairs], dt.int32, isOutput=False
    )
    c_traj = nc.declare_dram_parameter("c_traj", [seq * 128, 64], dt.float32, isOutput=True)
    h_traj = nc.declare_dram_parameter("h_traj", [seq * 128, 64], dt.float16, isOutput=True)

    with tile.TileContext(nc) as tc, ExitStack() as ctx:
        pool = lambda name, bufs, **kw: ctx.enter_context(
            tc.tile_pool(name=name, bufs=bufs, **kw)
        )
        whp = pool("whp", 1)
        tokp = pool("tokp", 1)
        stp = pool("stp", 1)
        hp = pool("hp", 1)
        cp = pool("cp", 1)
        # one PSUM pool per gate region; bank-granular alloc => 8 banks total
        zpools = [pool(f"z{r}", 2, space="PSUM") for r in range(4)]
        tgp = pool("tgp", 2)
        sip = pool("sip", 2)
        sfp = pool("sfp", 2)
        sop = pool("sop", 2)
        ap_ = pool("ap", 2)
        bp = pool("bp", 2)
        tp = pool("tp", 2)

        wh_sb = whp.tile([128, 64 * 128], WH_DT, name="wh_sb")
        nc.sync.dma_start(out=wh_sb[:], in_=wh[:, :])
        tok_cur = tokp.tile([32, pairs], dt.int32, name="tok_cur")
        nc.sync.dma_start(out=tok_cur[:], in_=tok2[:, 0:pairs])
        id_sb = tokp.tile([32, 32], dt.float16, name="id_sb")
        nc.sync.dma_start(out=id_sb[:], in_=ident[:, :])

        ST2 = [
            stp.tile([32, GATES], dt.float16, name=f"st{m}", tag=f"st{m}")
            for m in range(pairs)
        ]
        H = [hp.tile([128, 64], dt.float16, name=f"h{s}", tag=f"h{s}") for s in range(body)]
        C = [cp.tile([128, 64], dt.float32, name=f"c{s}", tag=f"c{s}") for s in range(body)]

        nc.gpsimd.memset(H[body - 1][:], 0.0)
        nc.gpsimd.memset(C[body - 1][:], 0.0)
        for m in range(pairs):
            # init shadow coverage; real values come from the indirect gathers
            nc.gpsimd.memset(ST2[m][:], 0.0)

        def gather_xp(m):
            # Gather 32 wi rows (2 steps x 16 sequences) into ST2[m][q, :] --
            # row-per-partition, the DGE-supported shape. tok_cur always holds
            # the token columns for the body being prefetched.
            nc.gpsimd.indirect_dma_start(
                out=ST2[m][:],
                out_offset=None,
                in_=wi[:, :],
                in_offset=bass.IndirectOffsetOnAxis(ap=tok_cur[:, m : m + 1], axis=0),
            )

        def alloc_z(par):
            return [
                zpools[r].tile([128, 64], dt.float32, name=f"z{r}_{par}", tag=f"z{r}")
                for r in range(4)
            ]

        def xpair(m, zt_even, zt_odd):
            # Transpose the gathered rows of both steps of pair slot m into
            # their PSUM region tiles via PE identity matmuls; one LDWEIGHTS
            # ([32, 128] chunk) feeds both steps' matmuls.
            for cch in range(16):
                r, j = cch // 4, cch % 4
                lhs = ST2[m][:, 128 * cch : 128 * cch + 128]
                # start=True clears has_written for the WHOLE bank, so only
                # the first matmul touching each region tile may set it.
                nc.tensor.matmul(
                    out=zt_even[r][:, 16 * j : 16 * j + 16],
                    lhsT=lhs,
                    rhs=id_sb[:, 0:16],
                    start=(j == 0),
                    stop=False,
                    skip_group_check=True,
                )
                nc.tensor.matmul(
                    out=zt_odd[r][:, 16 * j : 16 * j + 16],
                    lhsT=lhs,
                    rhs=id_sb[:, 16:32],
                    start=(j == 0),
                    stop=False,
                    skip_group_check=True,
                )

        def hblock(s, zt):
            hprev = H[(s - 1) % body]
            for r in range(4):
                for j in range(4):
                    for k in range(4):
                        nc.tensor.matmul(
                            out=zt[r][:, 16 * j : 16 * j + 16],
                            lhsT=wh_sb[:, ((r * 4 + j) * 4 + k) * 128 : ((r * 4 + j) * 4 + k) * 128 + 128],
                            rhs=hprev[:, 16 * k : 16 * k + 16],
                            start=False,
                            stop=(k == 3),
                            skip_group_check=True,
                        )

        inv_s = 1.0 / WSCALE

        def chain(iv2, s, zt):
            TG = tgp.tile([128, 64], dt.float16, name="TG", tag="TG")
            nc.scalar.activation(out=TG[:], in_=zt[0][:], func=Act.Tanh, scale=inv_s)
            SI = sip.tile([128, 64], dt.float16, name="SI", tag="SI")
            nc.scalar.activation(out=SI[:], in_=zt[1][:], func=Act.Sigmoid, scale=inv_s)
            SF = sfp.tile([128, 64], dt.float16, name="SF", tag="SF")
            nc.scalar.activation(out=SF[:], in_=zt[2][:], func=Act.Sigmoid, scale=inv_s)
            SO = sop.tile([128, 64], dt.float16, name="SO", tag="SO")
            nc.scalar.activation(out=SO[:], in_=zt[3][:], func=Act.Sigmoid, scale=inv_s)
            A = ap_.tile([128, 64], dt.float16, name="A", tag="A")
            nc.vector.tensor_tensor(out=A[:], in0=SI[:], in1=TG[:], op=Alu.mult)
            B = bp.tile([128, 64], dt.float32, name="B", tag="B")
            nc.vector.tensor_tensor(out=B[:], in0=SF[:], in1=C[(s - 1) % body][:], op=Alu.mult)
            cs = C[s]
            nc.vector.tensor_tensor(out=cs[:], in0=A[:], in1=B[:], op=Alu.add)
            T = tp.tile([128, 64], dt.float16, name="T", tag="T")
            nc.scalar.activation(out=T[:], in_=cs[:], func=Act.Tanh)
            hs = H[s]
            nc.vector.tensor_tensor(out=hs[:], in0=SO[:], in1=T[:], op=Alu.mult)

            nc.sync.dma_start(out=c_traj[ds((iv2 * 2 + s) * 128, 128), :], in_=cs[:])
            nc.sync.dma_start(out=h_traj[ds((iv2 * 2 + s) * 128, 128), :], in_=hs[:])

        # Preamble: gather all pair slots for body 0.
        for m in range(pairs):
            gather_xp(m)

        with tc.For_i(
            0, seq // 2, pairs, hint_engines=(mybir.EngineType.PE,), staggered_reset=True
        ) as iv2:
            # Stage the NEXT body's token columns; in-loop gathers prefetch
            # for body i+1 while this body computes (tok2 is padded).
            nc.sync.dma_start(out=tok_cur[:], in_=tok2[:, ds(iv2 + pairs, pairs)])
            for p in range(pairs):
                s0, s1 = 2 * p, 2 * p + 1
                # The x transposes sit between the previous pair's h-block and
                # this pair's on the PE queue, filling the tail-chain window.
                zcur = (alloc_z("e"), alloc_z("o"))
                xpair(p, *zcur)
                hblock(s0, zcur[0])
                chain(iv2, s0, zcur[0])
                hblock(s1, zcur[1])
                gather_xp(p)
                chain(iv2, s1, zcur[1])

    nc.finalize()
    return nc


def _get_program():
    global _PROGRAM
    if _PROGRAM is None:
        _PROGRAM = _build_program()
    return _PROGRAM


def _prep_host(inputs, Wi, Wh, b):
    tokens = np.argmax(inputs, axis=-1).astype(np.int32)  # [B, T]
    eos = inputs[:, :, EOS_ID] > 0.5
    any_eos = eos.any(axis=1)
    t_star = np.where(any_eos, eos.argmax(axis=1), SEQ - 1).astype(np.int64)

    # Gate reorder (g, i, f, o): tanh region first, sigma(o) last so only the
    # o-path trails the matmul block.
    perm = np.concatenate(
        [np.arange(1024, 1536), np.arange(0, 512), np.arange(512, 1024), np.arange(1536, 2048)]
    )
    Wi_re = (Wi.astype(np.float32) + b.astype(np.float32)[None, :])[:, perm] * WSCALE
    Wh_re = Wh.astype(np.float32)[:, perm] * WSCALE

    Wi_dev = np.ascontiguousarray(Wi_re).astype(np.float16)
    # wh[kr, ((r*4+j)*4+k)*128 + p] = Wh_re[128k+kr, 512r+128j+p]
    W5 = Wh_re.reshape(4, 128, 4, 4, 128)  # [k, kr, r, j, p]
    Wh_dev = np.ascontiguousarray(
        W5.transpose(1, 2, 3, 0, 4).reshape(128, 64 * 128)
    ).astype(WH_NP)
    return tokens, t_star, Wi_dev, Wh_dev


def kernel(inputs, Wi, Wh, b):
    global LAST_RESULTS
    inputs = np.asarray(inputs)
    Wi = np.asarray(Wi)
    Wh = np.asarray(Wh)
    b = np.asarray(b)

    tokens, t_star, Wi_dev, Wh_dev = _prep_host(inputs, Wi, Wh, b)

    in_maps = []
    for n in range(NCORES):
        tokc = tokens[BLOC * n : BLOC * (n + 1)]  # [16, 256]
        tk = tokc.reshape(BLOC, SEQ // 2, 2)
        tok2 = np.concatenate([tk[:, :, 0], tk[:, :, 1]], axis=0)  # [32, 128]
        tok2 = np.concatenate([tok2, np.zeros((32, PAIRS), np.int32)], axis=1)
        in_maps.append(
            {
                "wi": Wi_dev,
                "wh": Wh_dev,
                "tok2": np.ascontiguousarray(tok2),
                "ident": np.eye(32, dtype=np.float16),
            }
        )

    nc = _get_program()
    res = run_bass_kernel_spmd(nc, in_maps, list(range(NCORES)), trace=TRACE)
    LAST_RESULTS = res

    c_out = np.zeros((BATCH, HID), np.float32)
    h_out = np.zeros((BATCH, HID), np.float32)
    for n in range(NCORES):
        ct = res.results[n]["c_traj"].reshape(SEQ, 128, 64)
        ht = res.results[n]["h_traj"].reshape(SEQ, 128, 64).astype(np.float32)
        for bl in range(BLOC):
            g = BLOC * n + bl
            t = int(t_star[g])
            c_out[g] = ct[t][:, bl::BLOC].T.reshape(HID)
            h_out[g] = ht[t][:, bl::BLOC].T.reshape(HID)
    return (c_out, h_out)


# revision 9
# speedup vs baseline: 1.0209x; 1.0209x over previous
"""LSTM encoder with EOS-freeze for Trainium2, data-parallel over batch on 8 cores.

Strategy (v3)
-------------
Inputs are one-hot, so x @ Wi is a row-gather of Wi done with indirect DMA on
device; rows for two consecutive steps are gathered into one [32, 2048] tile so
one LDWEIGHTS transposes both steps (two matmuls against stacked identities).

The recurrent h @ Wh runs with Wh as 64 fp16 [128,128] stationary tiles and h
chunks as the [128,16] moving operand. The LDWEIGHTS+MATMUL pair pitch is
~27 ns regardless of dtype (the weight port moves columns, not bytes), so the
h-block costs ~1.7 us/step; everything else is arranged to overlap it.

Gates live in FOUR persistent PSUM tiles per step parity (8 banks: g,i,f,o x
even,odd), each closed by its own `stop` matmul, so every sigmoid/tanh fires
as soon as its own region finishes. Region order g,i,f,o leaves only
sigmoid(o) -> h on the post-block critical path. The x transposes for the
next pair are split: g/i chunks fill the even step's tail, f/o chunks the odd
step's tail.

The EOS freeze is handled without any per-step masking: sequences are
independent, so the kernel runs the unmasked recurrence and streams per-step
(c, h) snapshots to DRAM; the frozen value for sequence b is the snapshot at
its first-EOS step, selected during unshard.
"""

import numpy as np

try:
    import concourse  # noqa: F401
except ImportError:
    import sys

    sys.path.insert(0, "/opt/trn_rl_repo")

from contextlib import ExitStack

import concourse.bass as bass
import concourse.tile as tile
from concourse import bacc
from concourse import mybir
from concourse.bass import ds
from concourse.bass_utils import run_bass_kernel_spmd

dt = mybir.dt
Alu = mybir.AluOpType
Act = mybir.ActivationFunctionType

EOS_ID = 1
HID = 512
BATCH, SEQ, VOCAB = 128, 256, 1024
GATES = 4 * HID  # 2048
NCORES = 8
BLOC = BATCH // NCORES  # 16 sequences per core
BODY = 32  # steps per For_i iteration
PAIRS = BODY // 2  # x-gather/transpose granularity: 2 steps share one LDW

# Collect profiling info when True (set by test.py; adds trace overhead).
TRACE = False
LAST_RESULTS = None  # BassKernelResults of the last run, for test.py

_PROGRAM = None


def _build_program(seq=SEQ, body=BODY):
    pairs = body // 2
    nc = bacc.Bacc("TRN2", debug=False, detect_race_conditions=False)

    wi = nc.declare_dram_parameter("wi", [VOCAB, GATES], dt.float16, isOutput=False)
    ident = nc.declare_dram_parameter("ident", [32, 32], dt.float16, isOutput=False)
    wh = nc.declare_dram_parameter("wh", [128, 64 * 128], dt.float16, isOutput=False)
    tok2 = nc.declare_dram_parameter(
        "tok2", [32, seq // 2 + pairs], dt.int32, isOutput=False
    )
    c_traj = nc.declare_dram_parameter("c_traj", [seq * 128, 64], dt.float32, isOutput=True)
    h_traj = nc.declare_dram_parameter("h_traj", [seq * 128, 64], dt.float16, isOutput=True)

    with tile.TileContext(nc) as tc, ExitStack() as ctx:
        pool = lambda name, bufs, **kw: ctx.enter_context(
            tc.tile_pool(name=name, bufs=bufs, **kw)
        )
        whp = pool("whp", 1)
        tokp = pool("tokp", 1)
        stp = pool("stp", 1)
        hp = pool("hp", 1)
        cp = pool("cp", 1)
        zp = pool("zp", 1, space="PSUM")
        tgp = pool("tgp", 2)
        sip = pool("sip", 2)
        sfp = pool("sfp", 2)
        sop = pool("sop", 2)
        ap_ = pool("ap", 2)
        bp = pool("bp", 2)
        tp = pool("tp", 2)

        wh_sb = whp.tile([128, 64 * 128], dt.float16, name="wh_sb")
        nc.sync.dma_start(out=wh_sb[:], in_=wh[:, :])
        tok_cur = tokp.tile([32, pairs], dt.int32, name="tok_cur")
        nc.sync.dma_start(out=tok_cur[:], in_=tok2[:, 0:pairs])
        id_sb = tokp.tile([32, 32], dt.float16, name="id_sb")
        nc.sync.dma_start(out=id_sb[:], in_=ident[:, :])

        ST2 = [
            stp.tile([32, GATES], dt.float16, name=f"st{m}", tag=f"st{m}")
            for m in range(pairs)
        ]
        H = [hp.tile([128, 64], dt.float16, name=f"h{s}", tag=f"h{s}") for s in range(body)]
        C = [cp.tile([128, 64], dt.float32, name=f"c{s}", tag=f"c{s}") for s in range(body)]
        # Persistent per-parity gate region tiles: 4 regions x 2 parities = 8
        # PSUM banks. Region r of step s lives in Z[r][s % 2].
        Z = [
            [
                zp.tile([128, 64], dt.float32, name=f"z{r}{par}", tag=f"z{r}{par}")
                for par in range(2)
            ]
            for r in range(4)
        ]

        nc.gpsimd.memset(H[body - 1][:], 0.0)
        nc.gpsimd.memset(C[body - 1][:], 0.0)
        for m in range(pairs):
            # init shadow coverage; real values come from the indirect gathers
            nc.gpsimd.memset(ST2[m][:], 0.0)

        def gather_xp(m):
            # Gather 32 wi rows (2 steps x 16 sequences) into ST2[m][q, :] --
            # row-per-partition, the DGE-supported shape. tok_cur always holds
            # the token columns for the body being prefetched.
            nc.gpsimd.indirect_dma_start(
                out=ST2[m][:],
                out_offset=None,
                in_=wi[:, :],
                in_offset=bass.IndirectOffsetOnAxis(ap=tok_cur[:, m : m + 1], axis=0),
            )

        def xstep(s):
            # Transpose the gathered rows of step s (phase s%2 of pair slot
            # (s//2)%pairs) into its PSUM region tiles. start=True clears
            # has_written for the WHOLE bank, so only the first matmul
            # touching a region sets it. Emitted at the end of step s-2, so
            # these fill that step's tail-chain window on the PE queue.
            m = (s // 2) % pairs
            phase = s % 2
            par = s % 2
            for cch in range(16):
                r, j = cch // 4, cch % 4
                nc.tensor.matmul(
                    out=Z[r][par][:, 16 * j : 16 * j + 16],
                    lhsT=ST2[m][:, 128 * cch : 128 * cch + 128],
                    rhs=id_sb[:, 16 * phase : 16 * phase + 16],
                    start=(j == 0),
                    stop=False,
                    skip_group_check=True,
                )

        def hblock(s):
            par = s % 2
            hprev = H[(s - 1) % body]
            for r in range(4):
                for j in range(4):
                    for k in range(4):
                        t = (r * 4 + j) * 4 + k
                        nc.tensor.matmul(
                            out=Z[r][par][:, 16 * j : 16 * j + 16],
                            lhsT=wh_sb[:, t * 128 : t * 128 + 128],
                            rhs=hprev[:, 16 * k : 16 * k + 16],
                            start=False,
                            stop=(k == 3),
                            skip_group_check=True,
                        )

        def chain(iv2, s):
            par = s % 2
            TG = tgp.tile([128, 64], dt.float16, name="TG", tag="TG")
            nc.scalar.activation(out=TG[:], in_=Z[0][par][:], func=Act.Tanh)
            SI = sip.tile([128, 64], dt.float16, name="SI", tag="SI")
            nc.scalar.activation(out=SI[:], in_=Z[1][par][:], func=Act.Sigmoid)
            SF = sfp.tile([128, 64], dt.float16, name="SF", tag="SF")
            nc.scalar.activation(out=SF[:], in_=Z[2][par][:], func=Act.Sigmoid)
            SO = sop.tile([128, 64], dt.float16, name="SO", tag="SO")
            nc.scalar.activation(out=SO[:], in_=Z[3][par][:], func=Act.Sigmoid)
            A = ap_.tile([128, 64], dt.float16, name="A", tag="A")
            nc.vector.tensor_tensor(out=A[:], in0=SI[:], in1=TG[:], op=Alu.mult)
            B = bp.tile([128, 64], dt.float32, name="B", tag="B")
            nc.vector.tensor_tensor(out=B[:], in0=SF[:], in1=C[(s - 1) % body][:], op=Alu.mult)
            cs = C[s]
            nc.vector.tensor_tensor(out=cs[:], in0=A[:], in1=B[:], op=Alu.add)
            T = tp.tile([128, 64], dt.float16, name="T", tag="T")
            nc.scalar.activation(out=T[:], in_=cs[:], func=Act.Tanh)
            hs = H[s]
            nc.vector.tensor_tensor(out=hs[:], in0=SO[:], in1=T[:], op=Alu.mult)

            nc.sync.dma_start(out=c_traj[ds((iv2 * 2 + s) * 128, 128), :], in_=cs[:])
            nc.sync.dma_start(out=h_traj[ds((iv2 * 2 + s) * 128, 128), :], in_=hs[:])

        # Preamble: gather all pair slots for body 0, transpose steps 0/1.
        for m in range(pairs):
            gather_xp(m)
        xstep(0)
        xstep(1)

        with tc.For_i(
            0, seq // 2, pairs, hint_engines=(mybir.EngineType.PE,), staggered_reset=True
        ) as iv2:
            # Stage the NEXT body's token columns; in-loop gathers prefetch
            # for body i+1 while this body computes (tok2 is padded).
            nc.sync.dma_start(out=tok_cur[:], in_=tok2[:, ds(iv2 + pairs, pairs)])
            for p in range(pairs):
                s0, s1 = 2 * p, 2 * p + 1
                hblock(s0)
                chain(iv2, s0)
                # x for step s0+2 fills this step's tail-chain window
                # (step indices wrap into the next body via slot reuse).
                # Must be emitted AFTER chain(s0): the start=True write
                # invalidates the z this step's activations read.
                xstep((s0 + 2) % body)
                hblock(s1)
                gather_xp(p)
                chain(iv2, s1)
                xstep((s1 + 2) % body)

    nc.finalize()
    return nc


def _get_program():
    global _PROGRAM
    if _PROGRAM is None:
        _PROGRAM = _build_program()
    return _PROGRAM


def _prep_host(inputs, Wi, Wh, b):
    tokens = np.argmax(inputs, axis=-1).astype(np.int32)  # [B, T]
    eos = inputs[:, :, EOS_ID] > 0.5
    any_eos = eos.any(axis=1)
    t_star = np.where(any_eos, eos.argmax(axis=1), SEQ - 1).astype(np.int64)

    # Gate reorder (g, i, f, o): tanh region first, sigma(o) last so only the
    # o-path trails the matmul block.
    perm = np.concatenate(
        [np.arange(1024, 1536), np.arange(0, 512), np.arange(512, 1024), np.arange(1536, 2048)]
    )
    Wi_re = (Wi.astype(np.float32) + b.astype(np.float32)[None, :])[:, perm]
    Wh_re = Wh.astype(np.float32)[:, perm]

    Wi_dev = np.ascontiguousarray(Wi_re).astype(np.float16)
    # wh[kr, ((r*4+j)*4+k)*128 + p] = Wh_re[128k+kr, 512r+128j+p]
    W5 = Wh_re.reshape(4, 128, 4, 4, 128)  # [k, kr, r, j, p]
    Wh_dev = np.ascontiguousarray(
        W5.transpose(1, 2, 3, 0, 4).reshape(128, 64 * 128)
    ).astype(np.float16)
    return tokens, t_star, Wi_dev, Wh_dev


def kernel(inputs, Wi, Wh, b):
    global LAST_RESULTS
    inputs = np.asarray(inputs)
    Wi = np.asarray(Wi)
    Wh = np.asarray(Wh)
    b = np.asarray(b)

    tokens, t_star, Wi_dev, Wh_dev = _prep_host(inputs, Wi, Wh, b)

    in_maps = []
    for n in range(NCORES):
        tokc = tokens[BLOC * n : BLOC * (n + 1)]  # [16, 256]
        tk = tokc.reshape(BLOC, SEQ // 2, 2)
        tok2 = np.concatenate([tk[:, :, 0], tk[:, :, 1]], axis=0)  # [32, 128]
        tok2 = np.concatenate([tok2, np.zeros((32, PAIRS), np.int32)], axis=1)
        in_maps.append(
            {
                "wi": Wi_dev,
                "wh": Wh_dev,
                "tok2": np.ascontiguousarray(tok2),
                "ident": np.eye(32, dtype=np.float16),
            }
        )

    nc = _get_program()
    res = run_bass_kernel_spmd(nc, in_maps, list(range(NCORES)), trace=TRACE)
    LAST_RESULTS = res

    c_out = np.zeros((BATCH, HID), np.float32)
    h_out = np.zeros((BATCH, HID), np.float32)
    for n in range(NCORES):
        ct = res.results[n]["c_traj"].reshape(SEQ, 128, 64)
        ht = res.results[n]["h_traj"].reshape(SEQ, 128, 64).astype(np.float32)
        for bl in range(BLOC):
            g = BLOC * n + bl
            t = int(t_star[g])
            c_out[g] = ct[t][:, bl::BLOC].T.reshape(HID)
            h_out[g] = ht[t][:, bl::BLOC].T.reshape(HID)
    return (c_out, h_out)
